# revision 6
# baseline (speedup 1.0000x reference)
"""AxialTransformerBlock TRN2 kernel v2 (8 cores, SPMD).

vs v1: bf16 GEMM datapath, weight streams sized for full DMA speed, x1/x2
resident in SBUF, QKV AllToAll split in two bf16 halves overlapped with
projection compute, per-channel A AllToAll overlapped with flash, flash runs
two head-pairs interleaved sharing [128,1024] exp tiles, Wo_t+MLP per channel
interleaved into the flash stream.

Head/feature permutations (host-side, folded into weight layouts) make every
A2A scatter/gather a handful of <=3-dim constant-stride DMAs:
 - temporal Q/K features: head h=ph*8+j -> evens kt=4ph+i//16, odds kt=
   4ph+2+i//16, rows 16j+(i%16); per-dst message = rows [16j,16j+16).
 - temporal V / Wo_t features: f = 128*j + 64*ph + hd.
"""

import os
import numpy as np
import ml_dtypes

N_CORES = 8
S, C, D = 4096, 4, 1024
SB = S // N_CORES          # 512 s-rows per core
TL = SB * C                # 2048 local tokens
H_T, HD_T = 16, 64
F_MLP = 4 * D
LN_EPS = 1e-5

_CACHE = {}
_MARKS = []


def _build_program():
    import concourse.bass as bass
    import concourse.bacc as bacc
    import concourse.tile as tile
    from concourse import mybir

    F32 = mybir.dt.float32
    F32R = mybir.dt.float32r
    BF16 = mybir.dt.bfloat16
    FP8 = mybir.dt.float8e4
    AF = mybir.ActivationFunctionType
    OP = mybir.AluOpType
    ts = bass.ts

    nc = bacc.Bacc("TRN2", target_bir_lowering=False, debug=False,
                   num_devices=N_CORES)

    def din(name, shape, dt=F32):
        return nc.dram_tensor(name, list(shape), dt, kind="ExternalInput").ap()

    xT = din("xT", [D, TL])
    wqc = din("wqc", [D, D], BF16)
    wkc = din("wkc", [D, D], BF16)
    wvc = din("wvc", [D, D], BF16)
    woc = din("woc", [D, D], BF16)
    wqt = din("wqt", [D, D], BF16)
    wkt = din("wkt", [D, D], BF16)
    wvt = din("wvt", [D, D], BF16)
    wot = din("wot", [D, D], BF16)
    w1 = din("w1", [D, F_MLP], BF16)
    w2 = din("w2", [F_MLP, D], BF16)
    gb_c = din("gb_c", [D, 2])
    gb_t = din("gb_t", [D, 2])
    gb_m = din("gb_m", [D, 2])
    b1v = din("b1v", [F_MLP, 1])
    b2v = din("b2v", [D, 1])
    ropes = din("ropes", [8, 128, 512])     # (tgt2, e2, cs2) x [128, 512]
    mkc4 = din("mkc4", [128, 512], BF16)
    mkt2 = din("mkt2", [128, 128], BF16)
    idm_d = din("idm", [128, 128], BF16)

    yT = nc.dram_tensor("yT", [D, TL], F32, kind="ExternalOutput").ap()

    dbg = os.environ.get("KDBG", "0") == "1"
    kindd = "ExternalOutput" if dbg else "Internal"
    x1d = nc.dram_tensor("x1d", [D, TL], BF16, kind=kindd).ap() if dbg else None
    x2d = nc.dram_tensor("x2d", [D, TL], BF16, kind=kindd).ap() if dbg else None

    # A2A buffers: per channel, QK bf16 (16*16*512) then V fp8 per dst
    QKSZ, VSZ = 16 * 16 * 512, 128 * 4 * 128
    a2aQVi = [nc.dram_tensor(f"a2aQVi{c}", [8, QKSZ + VSZ // 2], BF16).ap()
              for c in range(C)]
    a2aQVo = [nc.dram_tensor(f"a2aQVo{c}", [8, QKSZ + VSZ // 2], BF16).ap()
              for c in range(C)]
    a2aAi = [nc.dram_tensor(f"a2aAi{c}", [8, 2, 64, 512], BF16).ap()
             for c in range(C)]
    a2aAo = [nc.dram_tensor(f"a2aAo{c}", [8, 2, 64, 512], BF16).ap()
             for c in range(C)]

    RG = [list(range(N_CORES))]

    def kpe(w, k=8):  # [D_in, E] dram -> [128, k, E]
        return w.rearrange("(k p) e -> p k e", p=128)

    with tile.TileContext(nc) as tc:
        cst_cm = tc.tile_pool(name="cst", bufs=1)
        cst = cst_cm.__enter__()
        x1_cm = tc.tile_pool(name="x1p", bufs=1)
        x1p = x1_cm.__enter__()

        ones1f = cst.tile([128, 1], F32)
        nc.vector.memset(ones1f, 1.0)
        ones1 = ones1f.bitcast(F32R)
        eps1 = cst.tile([1, 1], F32)
        nc.vector.memset(eps1, LN_EPS)
        gbc_sb = cst.tile([128, 8, 2], F32)
        nc.sync.dma_start(out=gbc_sb, in_=gb_c.rearrange("(k p) t -> p k t", p=128))
        gbt_sb = cst.tile([128, 8, 2], F32)
        nc.sync.dma_start(out=gbt_sb, in_=gb_t.rearrange("(k p) t -> p k t", p=128))
        gbm_sb = cst.tile([128, 8, 2], F32)
        nc.sync.dma_start(out=gbm_sb, in_=gb_m.rearrange("(k p) t -> p k t", p=128))
        b1_sb = cst.tile([128, 32], F32)
        nc.sync.dma_start(out=b1_sb, in_=b1v.rearrange("(k p) o -> p (k o)", p=128))
        b2_sb = cst.tile([128, 8], F32)
        nc.sync.dma_start(out=b2_sb, in_=b2v.rearrange("(k p) o -> p (k o)", p=128))
        mkc_sb = cst.tile([128, 512], BF16)
        nc.sync.dma_start(out=mkc_sb, in_=mkc4)
        mkt_sb = cst.tile([128, 128], BF16)
        nc.sync.dma_start(out=mkt_sb, in_=mkt2)
        idm_sb = cst.tile([128, 128], BF16)
        nc.sync.dma_start(out=idm_sb, in_=idm_d)

        # residual stream, c-major: x1c[c] [128, 8dt, 512s] bf16
        x1c = [x1p.tile([128, 8, 512], BF16, tag=f"x1_{c}", name=f"x1_{c}")
               for c in range(C)]
        ones1b = cst.tile([128, 1], BF16)
        nc.vector.memset(ones1b, 1.0)

        def ln_stats(pool, psm, x_t, hs, width, ones, sq_eng, share_ps=False):
            """mean/var for x_t[:, :, hs:hs+512]; returns (ab, bb) [128,512]."""
            if share_ps:
                st1 = psm.tile([128, 512], F32, tag="ps", name="st1",
                               bufs=2)[0:1, :]
                st2 = psm.tile([128, 512], F32, tag="ps", name="st2",
                               bufs=2)[0:1, :]
            else:
                st1 = psm.tile([1, 512], F32, tag="st1", name="st1", bufs=1)
                st2 = psm.tile([1, 512], F32, tag="st2", name="st2", bufs=1)
            for kt in range(8):
                nc.tensor.matmul(st1, ones, x_t[:, kt, hs:hs + 512],
                                 start=(kt == 0), stop=(kt == 7))
            for kt in range(8):
                xsq = pool.tile([128, 512], F32R, tag="lnxsq", name="xsq",
                                bufs=3)
                sq_eng.tensor_tensor(xsq, x_t[:, kt, hs:hs + 512],
                                     x_t[:, kt, hs:hs + 512], OP.mult)
                nc.tensor.matmul(st2, ones1, xsq,
                                 start=(kt == 0), stop=(kt == 7))
            sc = pool.tile([1, 2048], F32, tag="lnsc", name="lnsc", bufs=1)
            mu, ex2 = sc[:, 0:512], sc[:, 512:1024]
            w1s, w2s = sc[:, 1024:1536], sc[:, 1536:2048]
            nc.vector.tensor_scalar_mul(mu, st1, 1.0 / D)
            nc.vector.tensor_scalar_mul(ex2, st2, 1.0 / D)
            nc.vector.tensor_tensor(w1s, mu, mu, OP.mult)
            nc.vector.tensor_tensor(w1s, ex2, w1s, OP.subtract)   # var
            nc.scalar.activation(w2s, w1s, AF.Sqrt, bias=eps1)    # sd
            nc.vector.reciprocal(w1s, w2s)                        # rs
            nc.vector.tensor_tensor(w2s, mu, w1s, OP.mult)        # bv
            rs, bv = w1s, w2s
            ab = pool.tile([128, 512], F32R, tag="lnab", name="ab")
            nc.gpsimd.partition_broadcast(ab, rs.bitcast(F32R))
            bb = pool.tile([128, 512], F32R, tag="lnbb", name="bb")
            nc.gpsimd.partition_broadcast(bb, bv.bitcast(F32R))
            return ab, bb

        def layernorm(pool, psm, x_t, gb_sb, n_bf, width, ones=None,
                      sq_eng=None, share_ps=False):
            """x_t [128,8,width] f32r/bf16 -> n_bf [128,8,width] bf16."""
            sq_eng = sq_eng or nc.gpsimd
            ones = ones if ones is not None else ones1
            for hs in range(0, width, 512):
                ab, bb = ln_stats(pool, psm, x_t, hs, width, ones, sq_eng,
                                  share_ps)
                for kt in range(8):
                    t1 = pool.tile([128, 512], F32R, tag="lnt1", name="t1",
                                   bufs=3)
                    nc.vector.tensor_tensor(t1, x_t[:, kt, hs:hs + 512], ab,
                                            OP.mult)
                    nc.vector.tensor_tensor(t1, t1, bb, OP.subtract)
                    nc.vector.tensor_scalar(n_bf[:, kt, hs:hs + 512], t1,
                                            gb_sb[:, kt, 0:1], gb_sb[:, kt, 1:2],
                                            OP.mult, OP.add)

        _MARKS.append(("A", len(nc.inst_map)))
        # ---------------- Phase A: channel attention ----------------
        with (tc.tile_pool(name="pa", bufs=2) as pa,
              tc.tile_pool(name="pa1", bufs=1) as pa1,
              tc.tile_pool(name="pa_ps", bufs=1, space="PSUM") as pa_ps):
            for ch in range(4):          # chunks of 512 s-major tokens
                x_p = pa1.tile([128, 8, 512], F32R, tag="x_p", name="x_p",
                               bufs=2)
                nc.sync.dma_start(
                    out=x_p,
                    in_=xT.bitcast(F32R).rearrange("(k p) t -> p k t", p=128)[:, :, ts(ch, 512)])
                x_r = x_p
                n_bf = pa1.tile([128, 8, 512], BF16, tag="n_bf", name="n_bf",
                                bufs=2)
                layernorm(pa, pa_ps, x_r, gbc_sb, n_bf, 512)

                q_bf = pa1.tile([128, 8, 512], BF16, tag="q_bf", name="q_bf")
                k_bf = pa1.tile([128, 8, 512], BF16, tag="k_bf", name="k_bf")
                for wd, dst in ((wqc, q_bf), (wkc, k_bf)):
                    for e2 in range(4):    # 2 et tiles per load
                        w_t = pa.tile([128, 8, 256], BF16, tag="wst",
                                      name="w_t", bufs=3)
                        nc.sync.dma_start(out=w_t,
                                          in_=kpe(wd)[:, :, ts(e2, 256)])
                        for ei in range(2):
                            et = e2 * 2 + ei
                            ps = pa_ps.tile([128, 512], F32, tag="ps",
                                            name="ps", bufs=4)
                            for kt in range(8):
                                nc.tensor.matmul(
                                    ps, w_t[:, kt, ts(ei, 128)],
                                    n_bf[:, kt, :],
                                    start=(kt == 0), stop=(kt == 7))
                            nc.scalar.activation(dst[:, et, :], ps, AF.Copy)
                # V token-major
                v_bf = pa1.tile([128, 4, 1024], BF16, tag="v_bf", name="v_bf")
                for ec in range(2):
                    wv_t = pa.tile([128, 8, 512], BF16, tag="wvst", name="wv_t",
                                   bufs=2)
                    nc.sync.dma_start(out=wv_t, in_=kpe(wvc)[:, :, ts(ec, 512)])
                    for tt in range(4):
                        psv = pa_ps.tile([128, 512], F32, tag="ps", name="psv",
                                         bufs=4)
                        for kt in range(8):
                            nc.tensor.matmul(psv, n_bf[:, kt, ts(tt, 128)],
                                             wv_t[:, kt, :],
                                             start=(kt == 0), stop=(kt == 7))
                        nc.scalar.activation(v_bf[:, tt, ts(ec, 512)], psv,
                                             AF.Copy)

                # attention (block-diag 4x4 over s-major tokens)
                aT_bf = pa1.tile([128, 8, 512], BF16, tag="aT_bf", name="aT_bf")
                for h in range(4):
                    ps_s = pa_ps.tile([128, 512], F32, tag="ps", name="ps_s",
                                      bufs=4)
                    for g in range(4):
                        for hf in range(2):
                            nc.tensor.matmul(
                                ps_s[:, ts(g, 128)],
                                q_bf[:, 2 * h + hf, ts(g, 128)],
                                k_bf[:, 2 * h + hf, ts(g, 128)],
                                start=(hf == 0), stop=(hf == 1))
                    pm_t = pa.tile([128, 512], BF16, tag="pm", name="pm")
                    nc.scalar.activation(pm_t, ps_s, AF.Exp, scale=1.0 / 16.0)
                    nc.vector.tensor_tensor(pm_t, pm_t, mkc_sb, OP.mult)
                    for g in range(4):
                        den = pa.tile([128, 1], F32, tag="den", name="den",
                                      bufs=4)
                        nc.vector.reduce_sum(den, pm_t[:, ts(g, 128)],
                                             axis=mybir.AxisListType.X)
                        rec = pa.tile([128, 1], F32, tag="rec", name="rec",
                                      bufs=4)
                        nc.vector.reciprocal(rec, den)
                        nc.vector.tensor_scalar_mul(pm_t[:, ts(g, 128)],
                                                    pm_t[:, ts(g, 128)], rec)
                    ps_t = pa_ps.tile([128, 512], BF16, tag="ps_t",
                                      name="ps_t", bufs=2)
                    for g in range(4):
                        nc.tensor.transpose(ps_t[:, ts(g, 128)],
                                            pm_t[:, ts(g, 128)], idm_sb)
                    pT = pa.tile([128, 512], BF16, tag="pT", name="pT")
                    nc.scalar.activation(pT, ps_t, AF.Copy)
                    for hf in range(2):
                        es = 2 * h + hf
                        ps_av = pa_ps.tile([128, 512], F32, tag="ps",
                                           name="ps_av", bufs=4)
                        for g in range(4):
                            nc.tensor.matmul(
                                ps_av[:, ts(g, 128)],
                                v_bf[:, g, ts(es, 128)],
                                pT[:, ts(g, 128)],
                                start=True, stop=True)
                        nc.scalar.activation(aT_bf[:, es, :], ps_av, AF.Copy)

                # Wo_c + residual into x1c (c-major)
                for e2 in range(4):
                    w_t = pa.tile([128, 8, 256], BF16, tag="wst", name="w_t",
                                  bufs=3)
                    nc.sync.dma_start(out=w_t, in_=kpe(woc)[:, :, ts(e2, 256)])
                    for ei in range(2):
                        dt = e2 * 2 + ei
                        ps_o = pa_ps.tile([128, 512], F32, tag="ps",
                                          name="ps_o", bufs=4)
                        for et in range(8):
                            nc.tensor.matmul(ps_o, w_t[:, et, ts(ei, 128)],
                                             aT_bf[:, et, :],
                                             start=(et == 0), stop=(et == 7))
                        pso_c = ps_o.rearrange("p (s c) -> p c s", c=4)
                        xp_c = x_p[:, dt, :].rearrange("p (s c) -> p c s", c=4)
                        for c in range(4):
                            nc.vector.tensor_tensor(
                                x1c[c][:, dt, ts(ch, 128)],
                                pso_c[:, c, :], xp_c[:, c, :], OP.add)

        if dbg:
            for c in range(4):
                nc.sync.dma_start(
                    out=x1d.rearrange("d (c s) -> d c s", c=4)[:, c, :]
                    .rearrange("(k p) s -> p k s", p=128),
                    in_=x1c[c])

        _MARKS.append(("B", len(nc.inst_map)))
        # ---------------- Phase B: temporal QKV + scatter ----------------
        with (tc.tile_pool(name="pb", bufs=2) as pb,
              tc.tile_pool(name="pb1", bufs=1) as pb1,
              tc.tile_pool(name="pb_ps", bufs=1, space="PSUM") as pb_ps):
            rp_sb = pb1.tile([128, 8, 512], F32, tag="rope", name="rp_sb")
            nc.sync.dma_start(out=rp_sb, in_=ropes.rearrange("k p s -> p k s"))

            for c in range(C):
                half, ci = c // 2, c % 2
                x_r = x1c[c]
                n_bf = pb1.tile([128, 8, 512], BF16, tag="n_bf", name="n_bf",
                                bufs=2)
                layernorm(pb, pb_ps, x_r, gbt_sb, n_bf, 512, ones=ones1b)

                qk_bf = pb1.tile([128, 16, 512], BF16, tag="qk_bf",
                                 name="qk_bf", bufs=1)
                for tgt, wd in ((0, wqt), (1, wkt)):
                    for ph in range(2):
                        w_a = pb.tile([128, 8, 256], BF16, tag="wst",
                                      name="w_a", bufs=3)
                        nc.sync.dma_start(out=w_a,
                                          in_=kpe(wd)[:, :, ts(2 * ph, 256)])
                        w_b = pb.tile([128, 8, 256], BF16, tag="wst",
                                      name="w_b", bufs=3)
                        nc.sync.dma_start(out=w_b,
                                          in_=kpe(wd)[:, :, ts(2 * ph + 1, 256)])
                        for e in range(2):
                            ps_e = pb_ps.tile([128, 512], F32, tag="pse",
                                              name="ps_e", bufs=3)
                            ps_o = pb_ps.tile([128, 512], F32, tag="pso",
                                              name="ps_o", bufs=3)
                            for kt in range(8):
                                nc.tensor.matmul(ps_e, w_a[:, kt, ts(e, 128)],
                                                 n_bf[:, kt, :],
                                                 start=(kt == 0), stop=(kt == 7))
                            for kt in range(8):
                                nc.tensor.matmul(ps_o, w_b[:, kt, ts(e, 128)],
                                                 n_bf[:, kt, :],
                                                 start=(kt == 0), stop=(kt == 7))
                            cosT = rp_sb[:, tgt * 4 + e * 2, :].bitcast(F32R)
                            sinT = rp_sb[:, tgt * 4 + e * 2 + 1, :].bitcast(F32R)
                            t1 = pb.tile([128, 512], F32R, tag="rp1", name="t1")
                            t2 = pb.tile([128, 512], F32R, tag="rp2", name="t2")
                            t3 = pb.tile([128, 512], F32R, tag="rp3", name="t3")
                            t4 = pb.tile([128, 512], F32R, tag="rp4", name="t4")
                            nc.vector.tensor_tensor(t1, ps_e, cosT, OP.mult)
                            nc.vector.tensor_tensor(t2, ps_e, sinT, OP.mult)
                            nc.vector.tensor_tensor(t3, ps_o, sinT, OP.mult)
                            nc.vector.tensor_tensor(t4, ps_o, cosT, OP.mult)
                            kt_ev = tgt * 8 + 4 * ph + e
                            kt_od = tgt * 8 + 4 * ph + 2 + e
                            nc.gpsimd.tensor_tensor(qk_bf[:, kt_ev, :], t1, t3,
                                                    OP.subtract)
                            nc.gpsimd.tensor_tensor(qk_bf[:, kt_od, :], t2, t4,
                                                    OP.add)
                # V token-major, perm_v feature order
                v_bf = pb1.tile([128, 4, 1024], FP8, tag="v_bf", name="v_bf",
                                bufs=1)
                for ec in range(2):
                    wv_t = pb.tile([128, 8, 512], BF16, tag="wvst", name="wv_t",
                                   bufs=2)
                    nc.sync.dma_start(out=wv_t, in_=kpe(wvt)[:, :, ts(ec, 512)])
                    for tt in range(4):
                        psv = pb_ps.tile([128, 512], F32, tag="pse", name="psv",
                                         bufs=3)
                        for kt in range(8):
                            nc.tensor.matmul(psv, n_bf[:, kt, ts(tt, 128)],
                                             wv_t[:, kt, :],
                                             start=(kt == 0), stop=(kt == 7))
                        nc.scalar.activation(v_bf[:, tt, ts(ec, 512)], psv,
                                             AF.Copy)
                # scatter
                for j in range(8):
                    nc.sync.dma_start(
                        out=a2aQVi[c][j, 0:QKSZ].rearrange(
                            "(kt r s) -> r kt s", kt=16, r=16),
                        in_=qk_bf[16 * j:16 * (j + 1)])
                    nc.sync.dma_start(
                        out=a2aQVi[c][j, QKSZ:].bitcast(FP8).rearrange(
                            "(p tt f) -> p tt f", p=128, tt=4),
                        in_=v_bf[:, :, ts(j, 128)])
                if c < 3:
                    nc.gpsimd.collective_compute(
                        "AllToAll", OP.bypass, replica_groups=RG,
                        ins=[a2aQVi[c].opt()], outs=[a2aQVo[c].opt()])

        # ---------------- Flash + Wo_t + MLP, interleaved ----------------
        with (tc.tile_pool(name="pf", bufs=1) as pf,
              tc.tile_pool(name="pfw", bufs=2) as pfw,
              tc.tile_pool(name="pf_ps", bufs=1, space="PSUM") as pf_ps,
              tc.tile_pool(name="pm", bufs=2) as pm,
              tc.tile_pool(name="pm1", bufs=1) as pm1,
              tc.tile_pool(name="pm_ps", bufs=1, space="PSUM") as pm_ps):

            def flash_block(c):
                _MARKS.append((f"flash{c}", len(nc.inst_map)))
                # both head-pairs packed on partition halves of shared tiles
                kt2 = pf.tile([128, 8, 512], BF16, tag="kTp", name="kTp")
                qa2 = pf.tile([128, 8, 512], BF16, tag="qA", name="qA")
                kTp = [kt2[0:64], kt2[64:128]]
                qA = [qa2[0:64], qa2[64:128]]
                aTall = [pf.tile([64, 8, 512], BF16, tag=f"aT{ph}",
                                 name=f"aT{ph}") for ph in range(2)]
                vp = []
                for ph in range(2):
                    qkv = a2aQVo[c][:, 0:QKSZ].rearrange(
                        "src (kt r s) -> src kt r s", kt=16, r=16)
                    nc.scalar.dma_start(
                        out=kTp[ph],
                        in_=qkv[:, 8 + 4 * ph:8 + 4 * ph + 4]
                        .rearrange("src ktl r s -> (ktl r) src s"))
                    nc.scalar.dma_start(
                        out=qA[ph],
                        in_=qkv[:, 4 * ph:4 * ph + 4]
                        .rearrange("src ktl r s -> (ktl r) src s"))
                    vp_t = pf.tile([128, 32, 66], BF16, tag=f"vp{ph}",
                                   name=f"vp{ph}")
                    for src in range(8):
                        nc.gpsimd.dma_start(
                            out=vp_t[:, 4 * src:4 * src + 4, 0:64],
                            in_=a2aQVo[c][src, QKSZ:].bitcast(FP8).rearrange(
                                "(p tt f) -> p tt f", p=128, tt=4)
                            [:, :, 64 * ph:64 * ph + 64])
                    nc.vector.memset(vp_t[:, :, 64:65], 1.0)
                    vp.append(vp_t)
                for qc in range(8):
                    psa = [pf_ps.tile([128, 512], F32, tag=f"psa{ph}",
                                      name=f"psa{ph}", bufs=1)
                           for ph in range(2)]
                    nk = 4 * (qc + 1)
                    order = [4 * qc + i for i in range(4)] + list(range(4 * qc))
                    for idx, kt in enumerate(order):
                        diag = kt >= 4 * qc
                        o = 128 * (kt - 4 * qc) if diag else 0
                        ps2 = pf_ps.tile([128, 1024], F32, tag="ps2",
                                         name="ps2", bufs=2)
                        for ph in range(2):
                            nc.tensor.matmul(
                                ps2[:, 512 * ph + o:512 * ph + 512],
                                kTp[ph][:, kt // 4, ts(kt % 4, 128)],
                                qA[ph][:, qc, o:512], start=True, stop=True)
                        pexp = pfw.tile([128, 1024], BF16, tag="pexp",
                                        name="pexp", bufs=3)
                        if o == 0:
                            nc.scalar.activation(pexp, ps2, AF.Exp)
                        else:
                            for ph in range(2):
                                nc.scalar.activation(
                                    pexp[:, 512 * ph + o:512 * ph + 512],
                                    ps2[:, 512 * ph + o:512 * ph + 512],
                                    AF.Exp)
                        if diag:
                            for ph in range(2):
                                nc.vector.tensor_tensor(
                                    pexp[:, 512 * ph + o:512 * ph + o + 128],
                                    pexp[:, 512 * ph + o:512 * ph + o + 128],
                                    mkt_sb, OP.mult)
                        for ph in range(2):
                            nc.tensor.matmul(psa[ph][0:65, o:512],
                                             vp[ph][:, kt, 0:65],
                                             pexp[:, 512 * ph + o:512 * ph + 512],
                                             start=(idx == 0),
                                             stop=(idx == nk - 1),
                                             skip_group_check=True)
                    for ph in range(2):
                        rec1 = pfw.tile([1, 512], F32, tag="rec1", name="rec1")
                        nc.vector.reciprocal(rec1, psa[ph][64:65, :])
                        rb = pfw.tile([64, 512], F32R, tag="rb", name="rb")
                        nc.gpsimd.partition_broadcast(rb, rec1.bitcast(F32R))
                        nc.vector.tensor_tensor(aTall[ph][:, qc, :],
                                                psa[ph][0:64, :], rb, OP.mult)
                for ph in range(2):
                    nc.sync.dma_start(
                        out=a2aAi[c][:, ph].rearrange("qc hd s -> hd qc s"),
                        in_=aTall[ph])
                nc.gpsimd.collective_compute(
                    "AllToAll", OP.bypass, replica_groups=RG,
                    ins=[a2aAi[c].opt()], outs=[a2aAo[c].opt()])

            def bo_mlp(c):
                _MARKS.append((f"bomlp{c}", len(nc.inst_map)))
                # Wo_t + residual into x1c[c] (becomes x2)
                rhsA = pm1.tile([128, 8, 512], BF16, tag="rhsA", name="rhsA",
                                bufs=1)
                nc.sync.dma_start(
                    out=rhsA, in_=a2aAo[c].rearrange("j ph hd s -> (ph hd) j s"))
                for e2 in range(4):
                    w_t = pm.tile([128, 8, 256], BF16, tag="wst", name="w_t",
                                  bufs=2)
                    nc.sync.dma_start(out=w_t, in_=kpe(wot)[:, :, ts(e2, 256)])
                    for ei in range(2):
                        dt = e2 * 2 + ei
                        ps_o = pm_ps.tile([128, 512], F32, tag="ps", name="ps_o",
                                          bufs=2)
                        for et in range(8):
                            nc.tensor.matmul(ps_o, w_t[:, et, ts(ei, 128)],
                                             rhsA[:, et, :],
                                             start=(et == 0), stop=(et == 7))
                        nc.vector.tensor_tensor(x1c[c][:, dt, :], ps_o,
                                                x1c[c][:, dt, :], OP.add)
                if dbg:
                    nc.sync.dma_start(
                        out=x2d.rearrange("d (cc s) -> d cc s", cc=4)[:, c, :]
                        .rearrange("(k p) s -> p k s", p=128),
                        in_=x1c[c])
                # MLP
                x_r = x1c[c]
                n_bf = pm1.tile([128, 8, 512], BF16, tag="n_m", name="n_m",
                                bufs=1)
                layernorm(pm, pm_ps, x_r, gbm_sb, n_bf, 512, ones=ones1b,
                          share_ps=True)
                h_bf = pm1.tile([128, 32, 512], BF16, tag="h_bf", name="h_bf")
                for f2 in range(16):
                    w_t = pm.tile([128, 8, 256], BF16, tag="w1st", name="w1_t",
                                  bufs=2)
                    nc.sync.dma_start(out=w_t, in_=kpe(w1)[:, :, ts(f2, 256)])
                    for fi in range(2):
                        ft = f2 * 2 + fi
                        ps1 = pm_ps.tile([128, 512], F32, tag="ps", name="ps1",
                                         bufs=2)
                        for kt in range(8):
                            nc.tensor.matmul(ps1, w_t[:, kt, ts(fi, 128)],
                                             n_bf[:, kt, :],
                                             start=(kt == 0), stop=(kt == 7))
                        if ft % 2 == 0:
                            nc.scalar.activation(h_bf[:, ft, :], ps1, AF.Relu,
                                                 bias=b1_sb[:, ft:ft + 1])
                        else:
                            nc.vector.tensor_scalar(h_bf[:, ft, :], ps1,
                                                    b1_sb[:, ft:ft + 1], 0.0,
                                                    OP.add, OP.max)
                for dt in range(8):
                    w2_t = pm.tile([128, 32, 128], BF16, tag="w2st", name="w2_t",
                                   bufs=2)
                    nc.sync.dma_start(out=w2_t,
                                      in_=kpe(w2, k=32)[:, :, ts(dt, 128)])
                    if True:
                        ps_y = pm_ps.tile([128, 512], F32, tag="ps", name="ps_y",
                                          bufs=2)
                        for ft in range(32):
                            nc.tensor.matmul(ps_y, w2_t[:, ft, :],
                                             h_bf[:, ft, :],
                                             start=(ft == 0), stop=(ft == 31))
                        ty = pm.tile([128, 512], F32, tag="ty", name="ty")
                        nc.vector.tensor_tensor(ty, ps_y, x1c[c][:, dt, :],
                                                OP.add)
                        nc.vector.tensor_scalar_add(ty, ty,
                                                    b2_sb[:, dt:dt + 1])
                        nc.sync.dma_start(out=yT[ts(dt, 128), ts(c, 512)],
                                          in_=ty)

            flash_block(0)
            nc.gpsimd.collective_compute(
                "AllToAll", OP.bypass, replica_groups=RG,
                ins=[a2aQVi[3].opt()], outs=[a2aQVo[3].opt()])
            flash_block(1)
            flash_block(2)
            bo_mlp(0)
            flash_block(3)
            bo_mlp(1)
            bo_mlp(2)
            bo_mlp(3)

        x1_cm.__exit__(None, None, None)
        cst_cm.__exit__(None, None, None)

    nc.finalize()
    in_names = ["xT", "wqc", "wkc", "wvc", "woc", "wqt", "wkt", "wvt", "wot",
                "w1", "w2", "gb_c", "gb_t", "gb_m", "b1v", "b2v", "ropes",
                "mkc4", "mkt2", "idm"]
    return nc, in_names


def _host_prep(inputs):
    bf16 = ml_dtypes.bfloat16
    x = np.asarray(inputs["x"], np.float32)
    positions = np.asarray(inputs["positions"]).astype(np.int64)

    def T(a):
        return np.ascontiguousarray(np.asarray(a, np.float32).T)

    # temporal Q/K feature perm
    perm_qk = np.zeros(D, np.int64)
    for ph in range(2):
        for j in range(8):
            h = ph * 8 + j
            for i in range(32):
                f_ev = (4 * ph + i // 16) * 128 + 16 * j + (i % 16)
                f_od = (4 * ph + 2 + i // 16) * 128 + 16 * j + (i % 16)
                perm_qk[f_ev] = h * 64 + 2 * i
                perm_qk[f_od] = h * 64 + 2 * i + 1
    # temporal V / Wo_t feature perm
    perm_v = np.zeros(D, np.int64)
    for ph in range(2):
        for j in range(8):
            h = ph * 8 + j
            for hd in range(64):
                perm_v[128 * j + 64 * ph + hd] = h * 64 + hd

    def gb(g, b):
        return np.ascontiguousarray(
            np.stack([np.asarray(g, np.float32), np.asarray(b, np.float32)],
                     axis=1))

    shared = {
        "wqc": T(inputs["Wq_c"]).astype(bf16),
        "wkc": T(inputs["Wk_c"]).astype(bf16),
        "wvc": T(inputs["Wv_c"]).astype(bf16),
        "woc": T(inputs["Wo_c"]).astype(bf16),
        "wqt": np.ascontiguousarray(T(inputs["Wq_t"])[:, perm_qk]).astype(bf16),
        "wkt": np.ascontiguousarray(T(inputs["Wk_t"])[:, perm_qk]).astype(bf16),
        "wvt": np.ascontiguousarray(T(inputs["Wv_t"])[:, perm_v]).astype(bf16),
        "wot": np.ascontiguousarray(T(inputs["Wo_t"])[perm_v, :]).astype(bf16),
        "w1": T(inputs["W1"]).astype(bf16),
        "w2": T(inputs["W2"]).astype(bf16),
        "gb_c": gb(inputs["g_c"], inputs["b_c"]),
        "gb_t": gb(inputs["g_t"], inputs["b_t"]),
        "gb_m": gb(inputs["g_m"], inputs["b_m"]),
        "b1v": np.asarray(inputs["b1"], np.float32).reshape(F_MLP, 1),
        "b2v": np.asarray(inputs["b2"], np.float32).reshape(D, 1),
    }
    idx = np.arange(128)
    mkc = (idx[:, None] // 4 == idx[None, :] // 4).astype(np.float32)
    shared["mkc4"] = np.ascontiguousarray(
        np.tile(mkc, (1, 4))).astype(bf16)
    dk = np.arange(128)
    shared["mkt2"] = (dk[None, :] >= dk[:, None]).astype(np.float32).astype(bf16)
    shared["idm"] = np.eye(128, dtype=np.float32).astype(bf16)

    inv_freq = 10000.0 ** (-np.arange(32, dtype=np.float64) * 2 / HD_T)
    in_maps = []
    for i in range(N_CORES):
        m = dict(shared)
        xs = x[i * SB:(i + 1) * SB].reshape(TL, D)
        m["xT"] = np.ascontiguousarray(xs.T)
        pos = positions[i * SB:(i + 1) * SB].astype(np.float64)
        rp = np.zeros((8, 128, 512), np.float64)
        r = np.arange(128)
        for e in range(2):
            ivf = inv_freq[16 * e + (r % 16)]              # [128]
            ang = ivf[:, None] * pos[None, :]              # [128, 512]
            rp[0 + 2 * e] = np.cos(ang) * 0.125
            rp[1 + 2 * e] = np.sin(ang) * 0.125
            rp[4 + 2 * e] = np.cos(ang)
            rp[5 + 2 * e] = np.sin(ang)
        m["ropes"] = rp.astype(np.float32)
        in_maps.append(m)
    return in_maps


def _run(inputs, trace=False):
    from concourse.bass_utils import run_bass_kernel_spmd
    if "prog" not in _CACHE:
        _CACHE["prog"] = _build_program()
    nc, in_names = _CACHE["prog"]
    in_maps = _host_prep(inputs)
    for m in in_maps:
        for k in list(m.keys()):
            assert k in in_names, k
    res = run_bass_kernel_spmd(nc, in_maps, core_ids=list(range(N_CORES)),
                               trace=trace)
    out = np.zeros((S, C, D), np.float32)
    for i in range(N_CORES):
        yTl = res.results[i]["yT"]                       # [1024, 2048] c-major
        yi = yTl.T.reshape(C, SB, D)
        out[i * SB:(i + 1) * SB] = yi.transpose(1, 0, 2)
    return out, res


def kernel(**inputs) -> np.ndarray:
    out, _ = _run(inputs, trace=False)
    return out
